# revision 6
# baseline (speedup 1.0000x reference)
"""Pairwise Euclidean distance matrix on 8 TRN2 NeuronCores (Bass/Tile), v3.

out[i, j] = ||x[j] - x[i]||_2 for x [4096, 512] fp32.

Work decomposition (per core c, queries = block c of 512 points):
  - diag block (c,c): lower-left staircase only — query tile j (128 rows)
    vs keys 0..128*(j+1); host mirrors the rest. Keys come straight from
    the query SBUF tile (no separate upload).
  - ring blocks c+1..c+3: full [512 q x 512 k], host mirrors transpose.
  - ring block c+4: only half — cores c and c+4 split the unique content
    of the (c, c+4) block pair into complementary quadrant pairs. The
    quadrant choice is data-driven: cores >= 4 upload that key block with
    its two 256-column halves swapped, so the program is SPMD-uniform.
Total epilogue/output work: 16.5 tile-equivalents vs 20 for the naive
half-ring (the naive ring computes each distance-4 block twice).

Math: d2 = sq_q + sq_k - 2 x_q.x_k. The Gram part runs as fp8 e4m3
DoubleRow matmuls (2 k-tiles per instruction, 2x bf16 throughput);
queries are STATIONARY (loaded once per query tile, reused across all
key segments) and keys are MOVING. A tiny fp16 augmentation matmul
(contraction=2: [-sq_q/2; 1] x [1; -sq_k/2]) folds both squared-norm
terms into the same PSUM accumulation, so the epilogue is a single ACT
pass: out = Sqrt(-2 * psum), fp16, DMA'd out. fp8 Gram error ~0.1 abs
on d~32 against a 2e-2 * max|d| (~0.8) gate. The diagonal (d2 ~ 0) can
go slightly negative -> NaN from Sqrt; host zero-fills the diagonal.
"""

import numpy as np
import ml_dtypes

import concourse.bass as bass
import concourse.bacc as bacc
import concourse.tile as tile
from concourse.bass_utils import run_bass_kernel_spmd

mybir = bass.mybir

N = 4096
D = 512
NCORES = 8
QB = N // NCORES      # 512 queries per core
RKEYS = 4 * QB        # 2048 ring keys per core (blocks c+1..c+4)
AUGW = QB + QB + RKEYS  # aug table: [stationary 512 | diag-moving 512 | ring 2048]

_F8 = mybir.dt.float8e4
_F16 = mybir.dt.float16
_F32 = mybir.dt.float32

_NP8 = ml_dtypes.float8_e4m3

# DoubleRow fp8 (2 k-tiles per matmul) vs plain fp8 (4 k-tile matmuls).
# DR halves the matmul stream but serializes a 256-col LDWEIGHTS before
# every matmul (both per-cell weight regs are used by the interleave, so
# no background prefetch) — only wins if HAM promotes the clock anyway.
USE_DR = True

_nc_cache = {}


def _qt_layout(qt):
    """Per query-tile U2 segment packing: list of (kind, src_off, psum_off, w).

    kind: 'chunk2' (ring keys 1024:1536), 'diag' (keys = queries 0:w),
    'dist4' (ring keys 1536:1792 -> local 256-wide slice of block c+4).
    Segments are packed contiguously; every segment must stay inside one
    512-column PSUM bank, so the 256-wide dist4 segment is split when the
    staircase width would make it straddle a bank boundary.
    """
    # U2 = [chunk2 512 | diag staircase wd | dist4 256], hole-free, each
    # segment within one 512-col PSUM bank (dist4 split when needed).
    # dist4 src: query tiles 0,1 pair with the first uploaded 256 keys of
    # block c+4, tiles 2,3 with the second 256 (host swaps the halves for
    # cores >= 4 so the program stays SPMD-uniform).
    wd = 128 * (qt + 1)
    segs = [("chunk2", 1024, 0, 512), ("diag", 0, 512, wd)]
    off = 512 + wd
    d4_src = 1536 + (256 if qt >= 2 else 0)
    left = 256
    while left:
        room = 512 - (off % 512) if off % 512 else 512
        take = min(left, room)
        segs.append(("dist4", d4_src + 256 - left, off, take))
        off += take
        left -= take
    return segs, off


def _build():
    if "nc" in _nc_cache:
        return _nc_cache["nc"]
    nc = bacc.Bacc("TRN2", target_bir_lowering=False, debug=False)

    # fp8 keys (ring blocks, k-pair-major rows) / queries, fp16 aug table
    xk = nc.dram_tensor("xk", [D, RKEYS], _F8, kind="ExternalInput")
    xq = nc.dram_tensor("xq", [D, QB], _F8, kind="ExternalInput")
    aug = nc.dram_tensor("aug", [2, AUGW], _F16, kind="ExternalInput")
    # out rows = queries; scratch layout: cols 0:1024 = U1 (ring dist1,2),
    # cols 1024:1024+w2 = the per-qt U2 strip [chunk2 | diag | dist4]
    out = nc.dram_tensor("out", [QB, 2304], _F16, kind="ExternalOutput")

    # row d = 256*kp + 128*i + p  ->  [p, kp, i, n]
    xk4 = xk.ap().rearrange("(kp i p) n -> p kp i n", p=128, i=2)
    xq4 = xq.ap().rearrange("(kp i p) n -> p kp i n", p=128, i=2)

    DR = mybir.MatmulPerfMode.DoubleRow
    sqrtf = mybir.ActivationFunctionType.Sqrt

    with tile.TileContext(nc) as tc:
        with (
            tc.tile_pool(name="xd", bufs=1) as xd,
            tc.tile_pool(name="op", bufs=4) as op,
            # PSUM is 8 banks of 512 fp32 cols: U1 2 banks (x2),
            # U2 up to 3 banks (x1) = 7.
            tc.tile_pool(name="ps1", bufs=2, space="PSUM") as pp1,
            tc.tile_pool(name="ps2", bufs=1, space="PSUM") as pp2,
        ):
            # queries + aug first (every unit needs them; the diag units
            # need ONLY them, so they run while the key chunks stream in)
            t_q = xd.tile([128, 2, 2, QB], _F8, tag="q", name="q")
            nc.sync.dma_start(t_q[:], xq4)
            t_aug = xd.tile([2, AUGW], _F16, tag="aug", name="aug")
            nc.scalar.dma_start(t_aug[:], aug.ap())

            # ring key chunks of 512 on the sync/gpsimd queues (NOT scalar:
            # its ACT_TABLE_LOADs would sit between triggers and idle the
            # queue for ~2.6us right when chunk0 is needed)
            key_eng = [nc.gpsimd, nc.sync, nc.gpsimd, nc.gpsimd]
            t_key = []
            for ch in range(4):
                t = xd.tile([128, 2, 2, 512], _F8, tag=f"key{ch}", name=f"key{ch}")
                t_key.append(t)
            for ch in (0, 1, 2, 3):
                key_eng[ch].dma_start(
                    t_key[ch][:], xk4[:, :, :, ch * 512 : (ch + 1) * 512]
                )

            # HAM/pstate warmup: dense narrow matmuls until the first key
            # chunks land. HAM promotes during this burst and has
            # hysteresis — the real stream must start with no sparse gap
            # or it demotes for the rest of the kernel.
            warm = xd.tile([128, 64], _F8, tag="warm", name="warm")
            nc.vector.memset(warm[:], 0.0)
            wps = pp1.tile([64, 64], _F32, tag="u1", name="wps")
            for _ in range(80):
                nc.tensor.matmul(
                    wps[:], warm[:], warm[:, 0:64], start=True, stop=True
                )

            def ring_moving(kp, src_off, w):
                ch, o = src_off // 512, src_off % 512
                assert o + w <= 512
                return t_key[ch][:, kp, :, o : o + w]

            def gram_moving(kp, kind, src_off, w):
                if kind == "diag":
                    return t_q[:, kp, :, src_off : src_off + w]
                return ring_moving(kp, src_off, w)

            def aug_moving(kind, src_off, w):
                base = QB if kind == "diag" else 2 * QB
                return t_aug[:, base + src_off : base + src_off + w]

            dma_eng = [nc.sync, nc.scalar, nc.gpsimd]
            dma_i = [0]

            def dma_out(o_tile, o_off, qt, dst_off, w):
                dst = out.ap()[qt * 128 : (qt + 1) * 128, dst_off : dst_off + w]
                eng = dma_eng[dma_i[0] % 3]
                dma_i[0] += 1
                eng.dma_start(dst, o_tile[:, o_off : o_off + w])

            def run_unit(qt, segs, width, uname, pool, tag):
                """One PSUM unit: DR gram (2 passes) + fp16 aug, then ACT."""
                ps = pool.tile([128, width], _F32, tag=tag, name=f"ps{uname}")
                st_q = [t_q[:, kp, :, qt * 128 : (qt + 1) * 128] for kp in (0, 1)]
                st_a = t_aug[:, qt * 128 : (qt + 1) * 128]
                # PSUM start_tensor_calc marks a whole 2KB bank pending-zero,
                # so within a unit only the FIRST matmul touching each bank
                # may set start=True — a second start in the same bank wipes
                # the previous segment's partial sums. Later segments in an
                # already-started bank read pending-zero bytes as 0, which
                # accumulates correctly.
                started_banks = set()
                if USE_DR:
                    for kp in (0, 1):
                        for kind, so, po, w in segs:
                            bank = po // 512
                            assert (po + w - 1) // 512 == bank
                            st = kp == 0 and bank not in started_banks
                            if st:
                                started_banks.add(bank)
                            nc.tensor.matmul(
                                ps[:, po : po + w],
                                st_q[kp],
                                gram_moving(kp, kind, so, w),
                                start=st,
                                stop=False,
                                perf_mode=DR,
                            )
                else:
                    for kt in range(4):
                        kp, i = kt // 2, kt % 2
                        for kind, so, po, w in segs:
                            bank = po // 512
                            assert (po + w - 1) // 512 == bank
                            st = kt == 0 and bank not in started_banks
                            if st:
                                started_banks.add(bank)
                            mv = gram_moving(kp, kind, so, w)
                            nc.tensor.matmul(
                                ps[:, po : po + w],
                                st_q[kp][:, i, :],
                                mv[:, i, :],
                                start=st,
                                stop=False,
                            )
                for idx, (kind, so, po, w) in enumerate(segs):
                    nc.tensor.matmul(
                        ps[:, po : po + w],
                        st_a,
                        aug_moving(kind, so, w),
                        start=False,
                        stop=True,
                    )
                o = op.tile([128, width], _F16, tag="o", name=f"o{uname}")
                nc.scalar.activation(o[:], ps[:], sqrtf, bias=0.0, scale=-2.0)
                return o

            # qt0 has the smallest U2 strip — run it last so the final
            # ACT + out-DMA tail is as short as possible
            for qt in (1, 3, 2, 0):
                # U1: ring keys 0:1024 (dist-1 and dist-2 blocks)
                segs1 = [("ring", 0, 0, 512), ("ring", 512, 512, 512)]
                o1 = run_unit(qt, segs1, 1024, f"u1_{qt}", pp1, "u1")
                dma_out(o1, 0, qt, 0, 1024)
                # U2: chunk2 + diag staircase + dist4, one contiguous
                # strip per qt in the scratch out tensor (host unpacks)
                segs2, w2 = _qt_layout(qt)
                o2 = run_unit(qt, segs2, w2, f"u2_{qt}", pp2, "u2")
                dma_out(o2, 0, qt, 1024, w2)

    nc.compile()
    _nc_cache["nc"] = nc
    return nc


def _keycols(c):
    """Ring key global indices for core c: blocks c+1..c+3 natural, block
    c+4 with its 256-column halves swapped for cores >= 4."""
    cols = [np.arange(((c + t) % NCORES) * QB, ((c + t) % NCORES) * QB + QB)
            for t in (1, 2, 3)]
    b4 = ((c + 4) % NCORES) * QB
    d4 = np.arange(b4, b4 + QB)
    if c >= 4:
        d4 = np.concatenate([d4[256:], d4[:256]])
    cols.append(d4)
    return np.concatenate(cols)


def _prep_inputs(x: np.ndarray):
    x = np.ascontiguousarray(x, dtype=np.float32)
    x8 = x.astype(_NP8)
    x8T = np.ascontiguousarray(x8.T)  # [D, N]
    sqv = np.einsum("nd,nd->n", x.astype(np.float64), x.astype(np.float64))

    in_maps = []
    for c in range(NCORES):
        r0 = c * QB
        kc = _keycols(c)
        sq_q = sqv[r0 : r0 + QB]
        sq_k = sqv[kc]
        augm = np.empty((2, AUGW), dtype=np.float16)
        augm[0, :QB] = (-0.5 * sq_q).astype(np.float16)   # stationary row 0
        augm[1, :QB] = 1.0                                 # stationary row 1
        augm[0, QB : 2 * QB] = 1.0                         # diag moving row 0
        augm[1, QB : 2 * QB] = (-0.5 * sq_q).astype(np.float16)
        augm[0, 2 * QB :] = 1.0                            # ring moving row 0
        augm[1, 2 * QB :] = (-0.5 * sq_k).astype(np.float16)
        in_maps.append(
            {
                "xk": np.ascontiguousarray(x8T[:, kc]),
                "xq": np.ascontiguousarray(x8T[:, r0 : r0 + QB]),
                "aug": augm,
            }
        )
    return in_maps


def _assemble(results):
    full = np.empty((N, N), dtype=np.float32)
    for c in range(NCORES):
        O = results[c].astype(np.float32)  # [QB, 2304] scratch layout
        r0 = c * QB
        # ring blocks dist 1..3 + mirror: dist1,2 from U1 (cols 0:1024),
        # dist3 = chunk2 at cols 1024:1536 for every qt row block.
        for t, c0 in ((1, 0), (2, 512), (3, 1024)):
            b = ((c + t) % NCORES) * QB
            blk = O[:, c0 : c0 + QB]
            full[r0 : r0 + QB, b : b + QB] = blk
            full[b : b + QB, r0 : r0 + QB] = blk.T
        kc4 = _keycols(c)[3 * QB :]
        for j in range(4):
            wd = 128 * (j + 1)
            rs = slice(j * 128, (j + 1) * 128)
            qs = slice(r0 + j * 128, r0 + (j + 1) * 128)
            # diag staircase: query tile j vs keys 0..wd
            S = O[rs, 1536 : 1536 + wd]
            full[qs, r0 : r0 + wd] = S
            full[r0 : r0 + wd, qs] = S.T
            # dist4: tiles 0,1 -> uploaded keys 0:256, tiles 2,3 -> 256:512
            kg = kc4[0:256] if j < 2 else kc4[256:512]
            B4 = O[rs, 1536 + wd : 1792 + wd]
            full[qs, kg[0] : kg[0] + 256] = B4
            full[kg[0] : kg[0] + 256, qs] = B4.T
    np.fill_diagonal(full, 0.0)
    return full


def run(x: np.ndarray, trace: bool = False, tmpdir: str | None = None):
    nc = _build()
    in_maps = _prep_inputs(x)
    res = run_bass_kernel_spmd(
        nc, in_maps, list(range(NCORES)), trace=trace, tmpdir=tmpdir
    )
    full = _assemble([res.results[c]["out"] for c in range(NCORES)])
    return full, res


def kernel(x: np.ndarray) -> np.ndarray:
    out, _ = run(x, trace=False)
    return out
